# revision 4
# baseline (speedup 1.0000x reference)
"""Multi-head attention (B=2, S=2048, D=1024, H=16) on 8 Trainium2 cores.

Sharding: core c handles batch b = c//4 and head group g = c%4 (4 heads each).
Each core computes its heads' attention output and a partial output
projection [S, D]; the host sums the 4 partials per batch (the "all-reduce"
after W_o done host-side).

Device-kernel math per core (everything f32):
  Q.T = (s_b * W_q[rows]) @ X_b.T        [128=2heads*64, S] per head-pair
  K.T =  W_k[rows] @ X_b.T               (s_b = 1/8 folded into W_q on host)
  V   =  X_b @ W_v[rows].T               [S, 256] token-major
  S.T tile = K_tile @ Q.T                -> exp() -> E.T (no max-subtract:
       scores are O(5) so exp is safe in f32; softmax shift-invariance
       makes this equal to the reference up to rounding)
  PV: out[q,0:65] = sum_k E[q,k] * [V*z | z][k]  (ones-column trick:
       col 64 accumulates the softmax denominator; key-mask z zeroes
       masked keys' V rows so no mask pass over the S x S matrix)
  attn = num / den ; transpose via PE ; partial = attn @ W_o[:, rows].T
Edge case valid_len == 0: host sets s_b = 0 and z = ones -> E = 1
  -> uniform attention over all keys, exactly matching the reference.
"""

import sys

if "/opt/trn_rl_repo" not in sys.path:
    sys.path.insert(0, "/opt/trn_rl_repo")

import numpy as np
from contextlib import ExitStack

import concourse.bass as bass
import concourse.tile as tile
from concourse import bacc, mybir
from concourse import bass_utils
from concourse.masks import make_identity

F32 = mybir.dt.float32
EXP = mybir.ActivationFunctionType.Exp

B, S, D = 2, 2048, 1024
H, DK = 16, 64
HPC = 4            # heads per core
HC = HPC * DK      # head-group width = 256
N_CORES = 8
PT = 128           # partitions
NTT = S // PT      # 16 token tiles
NFC = D // PT      # 8 feature chunks
NQC = S // 512     # 4 q-chunks of 512


def _emit(tc, xt, wq, wk, wv, wo, zt, out):
    nc = tc.nc
    with ExitStack() as ctx:
        sb = ctx.enter_context(tc.tile_pool(name="sb", bufs=1))

        # ---- resident inputs ----
        xts = []
        for fc in range(NFC):
            t = sb.tile([PT, S], F32, name=f"xts{fc}")
            nc.sync.dma_start(t[:], xt[fc * PT:(fc + 1) * PT, :])
            xts.append(t)

        def load_w(ap, nm):
            ws = []
            for fc in range(NFC):
                t = sb.tile([PT, HC], F32, name=f"{nm}{fc}")
                nc.sync.dma_start(t[:], ap[fc * PT:(fc + 1) * PT, :])
                ws.append(t)
            return ws

        wqs, wks, wvs = load_w(wq, "wqs"), load_w(wk, "wks"), load_w(wv, "wvs")
        wos = []
        for c in range(2):
            t = sb.tile([PT, D], F32, name=f"wos{c}")
            nc.sync.dma_start(t[:], wo[c * PT:(c + 1) * PT, :])
            wos.append(t)
        ztt = sb.tile([PT, NTT], F32, name="ztt")
        nc.sync.dma_start(ztt[:], zt[:])
        ident = sb.tile([PT, PT], F32, name="ident")
        make_identity(nc, ident[:])

        # ---- resident intermediates ----
        qk_sb = {}
        for nm in ("q", "k"):
            for p in range(2):
                qk_sb[nm, p] = sb.tile([PT, S], F32, name=f"{nm}sb{p}")
        vzs = [sb.tile([PT, NTT, DK + 1], F32, name=f"vz{h}") for h in range(HPC)]
        attn = sb.tile([PT, NTT, HC], F32, name="attn")
        attnT = [sb.tile([PT, S], F32, name=f"attnT{c}") for c in range(2)]

        # ---- phase 1a: Q.T, K.T (2 heads stacked per 128-row tile) ----
        with tc.tile_pool(name="ps_qk", bufs=3, space="PSUM") as pq:
            for nm, ws in (("q", wqs), ("k", wks)):
                for p in range(2):
                    for qc in range(NQC):
                        pt = pq.tile([PT, 512], F32, name="pqk", tag="pqk")
                        for fc in range(NFC):
                            nc.tensor.matmul(
                                pt[:],
                                ws[fc][:, p * PT:(p + 1) * PT],
                                xts[fc][:, qc * 512:(qc + 1) * 512],
                                start=(fc == 0), stop=(fc == NFC - 1),
                            )
                        nc.vector.tensor_copy(
                            qk_sb[nm, p][:, qc * 512:(qc + 1) * 512], pt[:]
                        )

        # ---- phase 1b: V token-major, masked by z, plus ones(z) column ----
        with tc.tile_pool(name="ps_v", bufs=3, space="PSUM") as pv:
            for tt in range(NTT):
                pvt = pv.tile([PT, HC], F32, name="pvt", tag="pvt")
                for fc in range(NFC):
                    nc.tensor.matmul(
                        pvt[:],
                        xts[fc][:, tt * PT:(tt + 1) * PT],
                        wvs[fc][:],
                        start=(fc == 0), stop=(fc == NFC - 1),
                    )
                for h in range(HPC):
                    nc.vector.tensor_scalar_mul(
                        vzs[h][:, tt, 0:DK],
                        pvt[:, h * DK:(h + 1) * DK],
                        ztt[:, tt:tt + 1],
                    )
                    nc.vector.tensor_copy(
                        vzs[h][:, tt, DK:DK + 1], ztt[:, tt:tt + 1]
                    )

        # ---- phase 2: scores -> exp -> PV with ones-column denominator ----
        with tc.tile_pool(name="ps_s", bufs=2, space="PSUM") as pss, \
             tc.tile_pool(name="ps_a", bufs=1, space="PSUM") as psa, \
             tc.tile_pool(name="etp", bufs=4) as etp, \
             tc.tile_pool(name="rpp", bufs=8) as rpp:
            for h in range(HPC):
                po = (h % 2) * DK
                qa = qk_sb["q", h // 2][po:po + DK, :]
                ka = qk_sb["k", h // 2][po:po + DK, :]
                for qc in range(NQC):
                    pa = [
                        psa.tile([PT, DK + 1], F32, name=f"pa{qs}", tag=f"pa{qs}")
                        for qs in range(4)
                    ]
                    for kt in range(NTT):
                        st = pss.tile([PT, 512], F32, name="st", tag="st")
                        nc.tensor.matmul(
                            st[:],
                            ka[:, kt * PT:(kt + 1) * PT],
                            qa[:, qc * 512:(qc + 1) * 512],
                            start=True, stop=True,
                        )
                        et = etp.tile([PT, 512], F32, name="et", tag="et")
                        nc.scalar.activation(et[:], st[:], EXP)
                        for qs in range(4):
                            nc.tensor.matmul(
                                pa[qs][:],
                                et[:, qs * PT:(qs + 1) * PT],
                                vzs[h][:, kt, :],
                                start=(kt == 0), stop=(kt == NTT - 1),
                            )
                    for qs in range(4):
                        rc = rpp.tile([PT, 1], F32, name="rc", tag="rc")
                        nc.vector.reciprocal(rc[:], pa[qs][:, DK:DK + 1])
                        nc.vector.tensor_scalar_mul(
                            attn[:, qc * 4 + qs, h * DK:(h + 1) * DK],
                            pa[qs][:, 0:DK],
                            rc[:],
                        )

        # ---- phase 3: transpose attn [S, 256] -> attnT [256, S] on PE ----
        with tc.tile_pool(name="ps_t", bufs=4, space="PSUM") as pst:
            for tt in range(NTT):
                for c in range(2):
                    ptt = pst.tile([PT, PT], F32, name="ptt", tag="ptt")
                    nc.tensor.transpose(
                        ptt[:], attn[:, tt, c * PT:(c + 1) * PT], ident[:]
                    )
                    nc.vector.tensor_copy(
                        attnT[c][:, tt * PT:(tt + 1) * PT], ptt[:]
                    )

        # ---- phase 4: partial = attn @ W_o[:, rows].T ----
        with tc.tile_pool(name="ps_o", bufs=3, space="PSUM") as pso, \
             tc.tile_pool(name="stg", bufs=3) as stg:
            for tt in range(NTT):
                pot = pso.tile([PT, D], F32, name="pot", tag="pot")
                for half in range(2):
                    for c in range(2):
                        nc.tensor.matmul(
                            pot[:, half * 512:(half + 1) * 512],
                            attnT[c][:, tt * PT:(tt + 1) * PT],
                            wos[c][:, half * 512:(half + 1) * 512],
                            start=(c == 0), stop=(c == 1),
                        )
                so = stg.tile([PT, D], F32, name="so", tag="so")
                nc.vector.tensor_copy(so[:], pot[:])
                nc.sync.dma_start(out[tt * PT:(tt + 1) * PT, :], so[:])


def build():
    nc = bacc.Bacc(
        "TRN2",
        target_bir_lowering=False,
        debug=False,
        enable_asserts=True,
        num_devices=N_CORES,
    )
    xt = nc.dram_tensor("xt", [D, S], F32, kind="ExternalInput").ap()
    wq = nc.dram_tensor("wq", [D, HC], F32, kind="ExternalInput").ap()
    wk = nc.dram_tensor("wk", [D, HC], F32, kind="ExternalInput").ap()
    wv = nc.dram_tensor("wv", [D, HC], F32, kind="ExternalInput").ap()
    wo = nc.dram_tensor("wo", [HC, D], F32, kind="ExternalInput").ap()
    zt = nc.dram_tensor("zt", [PT, NTT], F32, kind="ExternalInput").ap()
    out = nc.dram_tensor("out", [S, D], F32, kind="ExternalOutput").ap()
    with tile.TileContext(nc) as tc:
        _emit(tc, xt, wq, wk, wv, wo, zt, out)
    nc.compile()
    return nc


_NC = None


def _get_nc():
    global _NC
    if _NC is None:
        _NC = build()
    return _NC


def make_in_maps(X, valid_lens, W_q, W_k, W_v, W_o):
    X = np.asarray(X, dtype=np.float32)
    W_q = np.asarray(W_q, dtype=np.float32)
    W_k = np.asarray(W_k, dtype=np.float32)
    W_v = np.asarray(W_v, dtype=np.float32)
    W_o = np.asarray(W_o, dtype=np.float32)
    vls = np.asarray(valid_lens).astype(np.int64)
    in_maps = []
    for c in range(N_CORES):
        b, g = divmod(c, 4)
        rows = slice(g * HC, (g + 1) * HC)
        vl = int(vls[b])
        s = 0.125 if vl > 0 else 0.0
        if vl > 0:
            z = (np.arange(S) < vl).astype(np.float32)
        else:
            z = np.ones(S, dtype=np.float32)
        in_maps.append({
            "xt": np.ascontiguousarray(X[b].T),
            "wq": np.ascontiguousarray(W_q[rows].T * s),
            "wk": np.ascontiguousarray(W_k[rows].T),
            "wv": np.ascontiguousarray(W_v[rows].T),
            "wo": np.ascontiguousarray(W_o.T[rows]),
            "zt": np.ascontiguousarray(z.reshape(NTT, PT).T),
        })
    return in_maps


def combine(outs):
    out = np.empty((B, S, D), dtype=np.float32)
    for b in range(B):
        out[b] = outs[4 * b] + outs[4 * b + 1] + outs[4 * b + 2] + outs[4 * b + 3]
    return out


def kernel(X, valid_lens, W_q, W_k, W_v, W_o):
    nc = _get_nc()
    in_maps = make_in_maps(X, valid_lens, W_q, W_k, W_v, W_o)
    res = bass_utils.run_bass_kernel_spmd(nc, in_maps, core_ids=list(range(N_CORES)))
    return combine([r["out"] for r in res.results])


# revision 7
# speedup vs baseline: 1.7128x; 1.7128x over previous
"""Multi-head attention (B=2, S=2048, D=1024, H=16) on 8 Trainium2 cores.

Sharding: core c handles batch b = c//4 and head group g = c%4 (4 heads each).
Each core computes its heads' attention output and a partial output
projection [S, D]; the host sums the 4 partials per batch (the "all-reduce"
after W_o done host-side).

Device-kernel math per core (everything f32):
  Q.T = (s_b * W_q[rows]) @ X_b.T        [128=2heads*64, S] per head-pair
  K.T =  W_k[rows] @ X_b.T               (s_b = 1/8 folded into W_q on host)
  V   =  X_b @ W_v[rows].T               [S, 256] token-major
  S.T tile = K_tile @ Q.T                -> exp() -> E.T (no max-subtract:
       scores are O(5) so exp is safe in f32; softmax shift-invariance
       makes this equal to the reference up to rounding)
  PV: out[q,0:65] = sum_k E[q,k] * [V*z | z][k]  (ones-column trick:
       col 64 accumulates the softmax denominator; key-mask z zeroes
       masked keys' V rows so no mask pass over the S x S matrix)
  attn = num / den ; transpose via PE ; partial = attn @ W_o[:, rows].T
Edge case valid_len == 0: host sets s_b = 0 and z = ones -> E = 1
  -> uniform attention over all keys, exactly matching the reference.
"""

import sys

if "/opt/trn_rl_repo" not in sys.path:
    sys.path.insert(0, "/opt/trn_rl_repo")

import numpy as np
from contextlib import ExitStack

import concourse.bass as bass
import concourse.tile as tile
from concourse import bacc, mybir
from concourse import bass_utils
from concourse.masks import make_identity

F32 = mybir.dt.float32
EXP = mybir.ActivationFunctionType.Exp

B, S, D = 2, 2048, 1024
H, DK = 16, 64
HPC = 4            # heads per core
HC = HPC * DK      # head-group width = 256
N_CORES = 8
PT = 128           # partitions
NTT = S // PT      # 16 token tiles
NFC = D // PT      # 8 feature chunks
NQC = S // 512     # 4 q-chunks of 512


def _emit_io_only(tc, xt, wq, wk, wv, wo, zt, out):
    # benchmarking aid: same I/O signature, no compute
    nc = tc.nc
    with ExitStack() as ctx:
        sb = ctx.enter_context(tc.tile_pool(name="sb", bufs=1))
        for fc in range(NFC):
            t = sb.tile([PT, S], F32, name=f"xts{fc}")
            nc.sync.dma_start(t[:], xt[fc * PT:(fc + 1) * PT, :])
        w = sb.tile([PT, HC], F32, name="w")
        nc.sync.dma_start(w[:], wq[0:PT, :])
        z = sb.tile([PT, D], F32, name="z")
        nc.vector.memset(z[:], 0.0)
        for tt in range(NTT):
            nc.sync.dma_start(out[tt * PT:(tt + 1) * PT, :], z[:])


def _emit(tc, xt, wq, wk, wv, wo, zt, out):
    nc = tc.nc
    with ExitStack() as ctx:
        sb = ctx.enter_context(tc.tile_pool(name="sb", bufs=1))

        # ---- resident inputs ----
        xts = []
        for fc in range(NFC):
            t = sb.tile([PT, S], F32, name=f"xts{fc}")
            nc.sync.dma_start(t[:], xt[fc * PT:(fc + 1) * PT, :])
            xts.append(t)

        def load_w(ap, nm):
            ws = []
            for fc in range(NFC):
                t = sb.tile([PT, HC], F32, name=f"{nm}{fc}")
                nc.sync.dma_start(t[:], ap[fc * PT:(fc + 1) * PT, :])
                ws.append(t)
            return ws

        wqs, wks, wvs = load_w(wq, "wqs"), load_w(wk, "wks"), load_w(wv, "wvs")
        wos = []
        for c in range(2):
            t = sb.tile([PT, D], F32, name=f"wos{c}")
            nc.sync.dma_start(t[:], wo[c * PT:(c + 1) * PT, :])
            wos.append(t)
        ztt = sb.tile([PT, NTT], F32, name="ztt")
        nc.sync.dma_start(ztt[:], zt[:])
        ident = sb.tile([PT, PT], F32, name="ident")
        make_identity(nc, ident[:])

        # ---- resident intermediates ----
        qk_sb = {}
        for nm in ("q", "k"):
            for p in range(2):
                qk_sb[nm, p] = sb.tile([PT, S], F32, name=f"{nm}sb{p}")
        vzs = [sb.tile([PT, NTT, DK + 1], F32, name=f"vz{h}") for h in range(HPC)]
        attn = sb.tile([PT, NTT, HC], F32, name="attn")
        attnT = [sb.tile([PT, S], F32, name=f"attnT{c}") for c in range(2)]

        # ---- phase 1a: Q.T, K.T (2 heads stacked per 128-row tile) ----
        with tc.tile_pool(name="ps_qk", bufs=3, space="PSUM") as pq:
            for nm, ws in (("q", wqs), ("k", wks)):
                for p in range(2):
                    for qc in range(NQC):
                        pt = pq.tile([PT, 512], F32, name="pqk", tag="pqk")
                        for fc in range(NFC):
                            nc.tensor.matmul(
                                pt[:],
                                ws[fc][:, p * PT:(p + 1) * PT],
                                xts[fc][:, qc * 512:(qc + 1) * 512],
                                start=(fc == 0), stop=(fc == NFC - 1),
                            )
                        nc.vector.tensor_copy(
                            qk_sb[nm, p][:, qc * 512:(qc + 1) * 512], pt[:]
                        )

        # ---- phase 1b: V token-major, masked by z, plus ones(z) column ----
        with tc.tile_pool(name="ps_v", bufs=3, space="PSUM") as pv:
            for tt in range(NTT):
                pvt = pv.tile([PT, HC], F32, name="pvt", tag="pvt")
                for fc in range(NFC):
                    nc.tensor.matmul(
                        pvt[:],
                        xts[fc][:, tt * PT:(tt + 1) * PT],
                        wvs[fc][:],
                        start=(fc == 0), stop=(fc == NFC - 1),
                    )
                for h in range(HPC):
                    nc.vector.tensor_scalar_mul(
                        vzs[h][:, tt, 0:DK],
                        pvt[:, h * DK:(h + 1) * DK],
                        ztt[:, tt:tt + 1],
                    )
                    nc.vector.tensor_copy(
                        vzs[h][:, tt, DK:DK + 1], ztt[:, tt:tt + 1]
                    )

        # ---- phase 2: scores -> exp -> PV with ones-column denominator ----
        with tc.tile_pool(name="ps_s", bufs=2, space="PSUM") as pss, \
             tc.tile_pool(name="ps_a", bufs=1, space="PSUM") as psa, \
             tc.tile_pool(name="etp", bufs=4) as etp, \
             tc.tile_pool(name="rpp", bufs=8) as rpp:
            for h in range(HPC):
                po = (h % 2) * DK
                qa = qk_sb["q", h // 2][po:po + DK, :]
                ka = qk_sb["k", h // 2][po:po + DK, :]
                for qc in range(NQC):
                    pa = [
                        psa.tile([PT, DK + 1], F32, name=f"pa{qs}", tag=f"pa{qs}")
                        for qs in range(4)
                    ]
                    for kt in range(NTT):
                        st = pss.tile([PT, 512], F32, name="st", tag="st")
                        nc.tensor.matmul(
                            st[:],
                            ka[:, kt * PT:(kt + 1) * PT],
                            qa[:, qc * 512:(qc + 1) * 512],
                            start=True, stop=True,
                        )
                        et = etp.tile([PT, 512], F32, name="et", tag="et")
                        nc.scalar.activation(et[:], st[:], EXP)
                        for qs in range(4):
                            nc.tensor.matmul(
                                pa[qs][:],
                                et[:, qs * PT:(qs + 1) * PT],
                                vzs[h][:, kt, :],
                                start=(kt == 0), stop=(kt == NTT - 1),
                            )
                    for qs in range(4):
                        rc = rpp.tile([PT, 1], F32, name="rc", tag="rc")
                        nc.vector.reciprocal(rc[:], pa[qs][:, DK:DK + 1])
                        nc.vector.tensor_scalar_mul(
                            attn[:, qc * 4 + qs, h * DK:(h + 1) * DK],
                            pa[qs][:, 0:DK],
                            rc[:],
                        )

        # ---- phase 3: transpose attn [S, 256] -> attnT [256, S] on PE ----
        with tc.tile_pool(name="ps_t", bufs=4, space="PSUM") as pst:
            for tt in range(NTT):
                for c in range(2):
                    ptt = pst.tile([PT, PT], F32, name="ptt", tag="ptt")
                    nc.tensor.transpose(
                        ptt[:], attn[:, tt, c * PT:(c + 1) * PT], ident[:]
                    )
                    nc.vector.tensor_copy(
                        attnT[c][:, tt * PT:(tt + 1) * PT], ptt[:]
                    )

        # ---- phase 4: partial = attn @ W_o[:, rows].T ----
        with tc.tile_pool(name="ps_o", bufs=3, space="PSUM") as pso, \
             tc.tile_pool(name="stg", bufs=3) as stg:
            for tt in range(NTT):
                pot = pso.tile([PT, D], F32, name="pot", tag="pot")
                for half in range(2):
                    for c in range(2):
                        nc.tensor.matmul(
                            pot[:, half * 512:(half + 1) * 512],
                            attnT[c][:, tt * PT:(tt + 1) * PT],
                            wos[c][:, half * 512:(half + 1) * 512],
                            start=(c == 0), stop=(c == 1),
                        )
                so = stg.tile([PT, D], F32, name="so", tag="so")
                nc.vector.tensor_copy(so[:], pot[:])
                nc.sync.dma_start(out[tt * PT:(tt + 1) * PT, :], so[:])


def build(io_only=False):
    nc = bacc.Bacc(
        "TRN2",
        target_bir_lowering=False,
        debug=False,
        enable_asserts=True,
        num_devices=N_CORES,
    )
    xt = nc.dram_tensor("xt", [D, S], F32, kind="ExternalInput").ap()
    wq = nc.dram_tensor("wq", [D, HC], F32, kind="ExternalInput").ap()
    wk = nc.dram_tensor("wk", [D, HC], F32, kind="ExternalInput").ap()
    wv = nc.dram_tensor("wv", [D, HC], F32, kind="ExternalInput").ap()
    wo = nc.dram_tensor("wo", [HC, D], F32, kind="ExternalInput").ap()
    zt = nc.dram_tensor("zt", [PT, NTT], F32, kind="ExternalInput").ap()
    out = nc.dram_tensor("out", [S, D], F32, kind="ExternalOutput").ap()
    with tile.TileContext(nc) as tc:
        (_emit_io_only if io_only else _emit)(tc, xt, wq, wk, wv, wo, zt, out)
    nc.compile()
    return nc


_NC = None


def _get_nc():
    global _NC
    if _NC is None:
        _NC = build()
    return _NC


def make_in_maps(X, valid_lens, W_q, W_k, W_v, W_o):
    X = np.asarray(X, dtype=np.float32)
    W_q = np.asarray(W_q, dtype=np.float32)
    W_k = np.asarray(W_k, dtype=np.float32)
    W_v = np.asarray(W_v, dtype=np.float32)
    W_o = np.asarray(W_o, dtype=np.float32)
    vls = np.asarray(valid_lens).astype(np.int64)
    in_maps = []
    for c in range(N_CORES):
        b, g = divmod(c, 4)
        rows = slice(g * HC, (g + 1) * HC)
        vl = int(vls[b])
        s = 0.125 if vl > 0 else 0.0
        if vl > 0:
            z = (np.arange(S) < vl).astype(np.float32)
        else:
            z = np.ones(S, dtype=np.float32)
        in_maps.append({
            "xt": np.ascontiguousarray(X[b].T),
            "wq": np.ascontiguousarray(W_q[rows].T * s),
            "wk": np.ascontiguousarray(W_k[rows].T),
            "wv": np.ascontiguousarray(W_v[rows].T),
            "wo": np.ascontiguousarray(W_o.T[rows]),
            "zt": np.ascontiguousarray(z.reshape(NTT, PT).T),
        })
    return in_maps


def combine(outs):
    out = np.empty((B, S, D), dtype=np.float32)
    for b in range(B):
        out[b] = outs[4 * b] + outs[4 * b + 1] + outs[4 * b + 2] + outs[4 * b + 3]
    return out


def kernel(X, valid_lens, W_q, W_k, W_v, W_o):
    nc = _get_nc()
    in_maps = make_in_maps(X, valid_lens, W_q, W_k, W_v, W_o)
    res = bass_utils.run_bass_kernel_spmd(nc, in_maps, core_ids=list(range(N_CORES)))
    return combine([r["out"] for r in res.results])
